# revision 6
# baseline (speedup 1.0000x reference)
"""Trainium2 Bass kernel for nn_BaselineGRU: 2-layer GRU (B=16,T=64,NN=4096,H=1024)
+ decoder, on 8 NeuronCores.

Strategy: gate-dimension sharding (each core owns a 128-wide hidden slice =
384 gate rows per layer), software-pipelined layers, one fused AllGather of
both layers' hidden-state chunks per pipeline slot.  All GEMMs keep the
stationary operand tiny (h.T chunks [128,16]) and stream weights as the
moving operand (fp32, N=384).  The layer-0 input projection is interleaved
into the PE stream so it hides under the early slots' AllGather windows.
"""
import numpy as np

import concourse.bacc as bacc
import concourse.tile as tile
import concourse.mybir as mybir
from concourse import bass_utils

B, T, NN, H = 16, 64, 4096, 1024
G3 = 3 * H                      # 3072 gate rows per layer
NC = 8                          # cores
HC = H // NC                    # 128 hidden per core
GC = 3 * HC                     # 384 gate rows per core
DC = NN // NC                   # 512 decoder rows per core
KH = H // 128                   # 8 K-chunks over hidden
KX = NN // 128                  # 32 K-chunks over input features
MT = (B * T) // 128             # 8 token blocks
fp32 = mybir.dt.float32
fp32r = mybir.dt.float32r
bf16 = mybir.dt.bfloat16       # PE: 1 cycle/moving-row for N>=256 (fp32: 4)

_CACHE = {}


def _build(debug=False, no_collective=False):
    nc = bacc.Bacc("TRN2", target_bir_lowering=False, debug=False,
                   enable_asserts=False, num_devices=NC)
    d = {}
    # ---- DRAM I/O ----
    d["xT"] = nc.dram_tensor("xT", [MT, 128, KX, 128], bf16, kind="ExternalInput").ap()
    d["wih0"] = nc.dram_tensor("wih0", [128, KX * GC], bf16, kind="ExternalInput").ap()
    d["whh0"] = nc.dram_tensor("whh0", [128, KH * GC], bf16, kind="ExternalInput").ap()
    d["eye128"] = nc.dram_tensor("eye128", [128, 128], fp32, kind="ExternalInput").ap()
    d["wih1"] = nc.dram_tensor("wih1", [128, KH * GC], bf16, kind="ExternalInput").ap()
    d["whh1"] = nc.dram_tensor("whh1", [128, KH * GC], bf16, kind="ExternalInput").ap()
    d["decw"] = nc.dram_tensor("decw", [128, KH * DC], bf16, kind="ExternalInput").ap()
    d["bih0"] = nc.dram_tensor("bih0", [1, GC], fp32r, kind="ExternalInput").ap()
    d["bhh0"] = nc.dram_tensor("bhh0", [1, GC], fp32r, kind="ExternalInput").ap()
    d["bih1"] = nc.dram_tensor("bih1", [1, GC], fp32r, kind="ExternalInput").ap()
    d["bhh1"] = nc.dram_tensor("bhh1", [1, GC], fp32r, kind="ExternalInput").ap()
    d["decb"] = nc.dram_tensor("decb", [1, DC], fp32r, kind="ExternalInput").ap()
    d["ones"] = nc.dram_tensor("ones", [1, 128], fp32r, kind="ExternalInput").ap()
    d["eye16"] = nc.dram_tensor("eye16", [16, 16], fp32, kind="ExternalInput").ap()
    out_d = nc.dram_tensor("out", [B, DC], fp32, kind="ExternalOutput").ap()
    if debug:
        dbg_gx0 = nc.dram_tensor("dbg_gx0", [T, B, GC], fp32, kind="ExternalOutput").ap()
        dbg_ag0 = nc.dram_tensor("dbg_ag0", [2 * 128 * NC, 16], fp32, kind="ExternalOutput").ap()
        dbg_ag1 = nc.dram_tensor("dbg_ag1", [2 * 128 * NC, 16], fp32, kind="ExternalOutput").ap()

    S = mybir.ActivationFunctionType.Sigmoid
    TA = mybir.ActivationFunctionType.Tanh

    with tile.TileContext(nc) as tc:
        with tc.tile_pool(name="wsb", bufs=1) as wsb, \
             tc.tile_pool(name="xp", bufs=2) as xp, \
             tc.tile_pool(name="gxp", bufs=3) as gxp, \
             tc.tile_pool(name="hp", bufs=2) as hp, \
             tc.tile_pool(name="gp", bufs=2) as gp, \
             tc.tile_pool(name="ps", bufs=1, space="PSUM") as ps, \
             tc.tile_pool(name="dram", bufs=1, space="DRAM") as drp, \
             tc.tile_pool(name="agd", bufs=16, space="DRAM") as agd:

            # ---- persistent SBUF loads ----
            wih0 = wsb.tile([128, KX * GC], bf16, tag="wih0")
            whh0 = wsb.tile([128, KH * GC], bf16, tag="whh0")
            wih1 = wsb.tile([128, KH * GC], bf16, tag="wih1")
            whh1 = wsb.tile([128, KH * GC], bf16, tag="whh1")
            decw = wsb.tile([128, KH * DC], bf16, tag="decw")
            bias = {}
            for nm in ("bih0", "bhh0", "bih1", "bhh1"):
                bias[nm] = wsb.tile([1, GC], fp32r, tag=nm, name=nm)
                nc.sync.dma_start(out=bias[nm][:], in_=d[nm])
            decb = wsb.tile([1, DC], fp32r, tag="decb")
            nc.sync.dma_start(out=decb[:], in_=d["decb"])
            ones = wsb.tile([1, 128], fp32r, tag="ones")
            nc.sync.dma_start(out=ones[:], in_=d["ones"])
            eye16 = wsb.tile([16, 16], fp32, tag="eye16")
            nc.sync.dma_start(out=eye16[:], in_=d["eye16"])
            eye128 = wsb.tile([128, 128], fp32, tag="eye128")
            nc.sync.dma_start(out=eye128[:], in_=d["eye128"])
            WQ = KX * GC // 4
            for q in range(4):
                nc.sync.dma_start(out=wih0[:, q * WQ:(q + 1) * WQ],
                                  in_=d["wih0"][:, q * WQ:(q + 1) * WQ])
            nc.sync.dma_start(out=whh0[:], in_=d["whh0"])
            nc.sync.dma_start(out=wih1[:], in_=d["wih1"])
            nc.sync.dma_start(out=whh1[:], in_=d["whh1"])
            nc.sync.dma_start(out=decw[:], in_=d["decw"])

            zero16 = wsb.tile([16, HC], fp32, tag="zero16")
            zeroT = wsb.tile([128, 16], fp32, tag="zeroT")
            nc.vector.memset(zeroT[:], 0.0)
            nc.vector.memset(zero16[:], 0.0)

            # gx0.T staging in DRAM: [MT, 3, 128 gates, 128 tokens]
            gx0_dram = drp.tile([MT, 3, 128, 128], fp32, tag="gx0d")

            # ---- layer-0 projection (transposed): out[g,tok] per gate-chunk
            def emit_proj_chunk(m):
                xtm = xp.tile([128, KX * 128], bf16, tag="xtm")
                nc.sync.dma_start(out=xtm[:].rearrange("p (k j) -> p k j", k=KX),
                                  in_=d["xT"][m])
                for g in range(3):
                    pp = ps.tile([128, 128], fp32, tag="proj")
                    for k in range(KX):
                        nc.tensor.matmul(pp[:],
                                         wih0[:, k * GC + g * 128:k * GC + (g + 1) * 128],
                                         xtm[:, k * 128:(k + 1) * 128],
                                         start=(k == 0), stop=False)
                    nc.tensor.matmul(pp[:], bias["bih0"][0:1, g * 128:(g + 1) * 128],
                                     ones[0:1, 0:128], start=False, stop=True)
                    psb = gxp.tile([128, 128], fp32, tag=f"projsb{g}")
                    psb_refs[(m, g)] = psb
                    nc.vector.tensor_copy(psb[:], pp[:])
                    nc.sync.dma_start(out=gx0_dram[m, g], in_=psb[:])

            # per-slot state
            h1T_prev = None      # [128, 128] SBUF: h1(s-1).T as 8 chunks of 16
            h2T_prev = None
            h1_old = zeroT       # [128,16] my slice of h1(s-1), transposed
            h2_old = zero16

            def gates(gh_ps, gx_ap, h_old, tagpfx):
                """gh_ps: PSUM [16,GC] (includes b_hh); gx_ap: [16,GC] (includes
                b_ih). Returns h_new [16,HC] SBUF tile."""
                pre = gp.tile([16, 2 * HC], fp32, tag=f"{tagpfx}pre")
                rz = gp.tile([16, 2 * HC], fp32, tag=f"{tagpfx}rz")
                c0 = gp.tile([16, HC], fp32, tag=f"{tagpfx}c0")
                d0 = gp.tile([16, HC], fp32, tag=f"{tagpfx}d0")
                n0 = gp.tile([16, HC], fp32, tag=f"{tagpfx}n0")
                e0 = gp.tile([16, HC], fp32, tag=f"{tagpfx}e0")
                f0 = gp.tile([16, HC], fp32, tag=f"{tagpfx}f0")
                hn = gp.tile([16, HC], fp32, tag=f"{tagpfx}hn")
                # pre_r,pre_z = gx[0:256] + gh[0:256]
                nc.vector.tensor_add(pre[:], gx_ap[:, 0:2 * HC], gh_ps[:, 0:2 * HC])
                nc.scalar.activation(rz[:], pre[:], S)
                # c = r * gh_n ; d = gx_n + c ; n = tanh(d)
                nc.vector.tensor_mul(c0[:], rz[:, 0:HC], gh_ps[:, 2 * HC:3 * HC])
                nc.vector.tensor_add(d0[:], gx_ap[:, 2 * HC:3 * HC], c0[:])
                nc.scalar.activation(n0[:], d0[:], TA)
                # h_new = n + z*(h_old - n)
                nc.vector.tensor_sub(e0[:], h_old[:], n0[:])
                nc.vector.tensor_mul(f0[:], rz[:, HC:2 * HC], e0[:])
                hnew = gp.tile([16, HC], fp32, tag=f"{tagpfx}hnew")
                nc.vector.tensor_add(hnew[:], n0[:], f0[:])
                return hnew

            proj_sched = {0: [0], 1: [1], 2: [2]}
            for j in range(5):
                proj_sched.setdefault(5 + j * 4, []).append(3 + j)
            psb_refs = {}

            for s in range(T + 1):
                for m in proj_sched.get(s, []):
                    emit_proj_chunk(m)

                # ---------- layer 0, step s (transposed gate layout) ----------
                if s < T:
                    # gx0.T for this step: [128, (g b)] = 3 gate-chunks x 16
                    if s == 0:
                        gxT = None
                        gx_sl = [psb_refs[(0, g)][:, 0:16] for g in range(3)]
                    else:
                        gxT = gxp.tile([128, 3 * 16], fp32, tag="gxT")
                        nc.sync.dma_start(
                            out=gxT[:].rearrange("p (g b) -> p g b", g=3),
                            in_=gx0_dram[s // 8, :, :,
                                         (s % 8) * 16:(s % 8) * 16 + 16]
                            .rearrange("g p b -> p g b"))
                        gx_sl = [gxT[:, g * 16:(g + 1) * 16] for g in range(3)]
                    # pre-activations assembled in PSUM: gh (+bhh) per chunk,
                    # plus gx injected for r,z (n keeps gx separate)
                    pch = []
                    for g in range(3):
                        pg = ps.tile([128, 16], fp32, tag=f"gh0T{g}")
                        first = True
                        if s > 0:
                            for k in range(KH):
                                nc.tensor.matmul(
                                    pg[:],
                                    whh0[:, k * GC + g * 128:k * GC + (g + 1) * 128],
                                    h1T_prev[:, k * 16:(k + 1) * 16],
                                    start=first, stop=False)
                                first = False
                        nc.tensor.matmul(pg[:],
                                         bias["bhh0"][0:1, g * 128:(g + 1) * 128],
                                         ones[0:1, 0:16],
                                         start=first, stop=True)
                        pch.append(pg)
                    prer = gp.tile([128, 16], fp32, tag="l0pr")
                    nc.vector.tensor_add(prer[:], gx_sl[0], pch[0][:])
                    prez = gp.tile([128, 16], fp32, tag="l0pz")
                    nc.vector.tensor_add(prez[:], gx_sl[1], pch[1][:])
                    r0 = gp.tile([128, 16], fp32, tag="l0r")
                    z0 = gp.tile([128, 16], fp32, tag="l0z")
                    nc.scalar.activation(r0[:], prer[:], S)
                    nc.scalar.activation(z0[:], prez[:], S)
                    c0 = gp.tile([128, 16], fp32, tag="l0c")
                    nc.vector.tensor_mul(c0[:], r0[:], pch[2][:])
                    d0 = gp.tile([128, 16], fp32, tag="l0d")
                    nc.vector.tensor_add(d0[:], gx_sl[2], c0[:])
                    n0 = gp.tile([128, 16], fp32, tag="l0n")
                    nc.scalar.activation(n0[:], d0[:], TA)
                    e0 = gp.tile([128, 16], fp32, tag="l0e")
                    nc.vector.tensor_sub(e0[:], h1_old[:], n0[:])
                    f0 = gp.tile([128, 16], fp32, tag="l0f")
                    nc.vector.tensor_mul(f0[:], z0[:], e0[:])
                    h1_new = gp.tile([128, 16], fp32, tag="l0h")
                    nc.vector.tensor_add(h1_new[:], n0[:], f0[:])
                    hbf = gp.tile([128, 16], bf16, tag="l0hb")
                    nc.vector.tensor_copy(hbf[:], h1_new[:])
                    agin0 = agd.tile([128, 16], bf16, tag="agin0")
                    nc.sync.dma_start(out=agin0[:], in_=hbf[:])
                    agout0 = agd.tile([128 * NC, 16], bf16, tag="agout0")
                    nc.gpsimd.collective_compute(
                        "AllGather", mybir.AluOpType.bypass,
                        replica_groups=[list(range(NC))],
                        ins=[agin0.opt()], outs=[agout0.opt()])
                    h1T = hp.tile([128, 128], bf16, tag="h1T")
                    nc.sync.dma_start(
                        out=h1T[:].rearrange("p (r b) -> p r b", r=8),
                        in_=agout0[:].rearrange("(r p) b -> p r b", p=128))
                    h1_old = h1_new

                # ---------- layer 1, step s-1 ----------
                if s >= 2:
                    # fetch h2(s-2).T gathered by last slot's AG1; its wait is
                    # already satisfied here, so no sequencer head-of-line block
                    h2T = hp.tile([128, 128], bf16, tag="h2T")
                    nc.scalar.dma_start(
                        out=h2T[:].rearrange("p (r b) -> p r b", r=8),
                        in_=agout1_prev[:].rearrange("(r p) b -> p r b", p=128))
                    h2T_prev = h2T
                if s >= 1:
                    gx1 = ps.tile([16, GC], fp32, tag="gx1")
                    for k in range(KH):
                        nc.tensor.matmul(gx1[:], h1T_prev[:, k * 16:(k + 1) * 16],
                                         wih1[:, k * GC:(k + 1) * GC],
                                         start=(k == 0), stop=False)
                    nc.tensor.matmul(gx1[:], ones[0:1, 0:16], bias["bih1"][:],
                                     start=False, stop=True)
                    gh1 = ps.tile([16, GC], fp32, tag="gh1")
                    if s == 1:
                        nc.tensor.matmul(gh1[:], ones[0:1, 0:16], bias["bhh1"][:],
                                         start=True, stop=True)
                    else:
                        for k in range(KH):
                            nc.tensor.matmul(gh1[:], h2T_prev[:, k * 16:(k + 1) * 16],
                                             whh1[:, k * GC:(k + 1) * GC],
                                             start=(k == 0), stop=False)
                        nc.tensor.matmul(gh1[:], ones[0:1, 0:16], bias["bhh1"][:],
                                         start=False, stop=True)
                    gx1s = gxp.tile([16, GC], fp32, tag="gx1s")
                    nc.vector.tensor_copy(gx1s[:], gx1[:])
                    h2_new = gates(gh1, gx1s[:], h2_old, "l1")
                    t1 = ps.tile([128, 16], fp32, tag="t1")
                    nc.tensor.transpose(t1[:], h2_new[:], eye16[:])
                    t1s = gp.tile([128, 16], bf16, tag="t1s")
                    nc.vector.tensor_copy(t1s[:], t1[:])
                    agin1 = agd.tile([128, 16], bf16, tag="agin1")
                    nc.scalar.dma_start(out=agin1[:], in_=t1s[:])
                    agout1 = agd.tile([128 * NC, 16], bf16, tag="agout1")
                    nc.gpsimd.collective_compute(
                        "AllGather", mybir.AluOpType.bypass,
                        replica_groups=[list(range(NC))],
                        ins=[agin1.opt()], outs=[agout1.opt()])
                    agout1_prev = agout1
                    h2_old = h2_new

                if s < T:
                    h1T_prev = h1T

            # ---------- decoder: out = h2(T-1) @ dec_w_c.T + dec_b_c ----------
            h2T_fin = hp.tile([128, 128], bf16, tag="h2T")
            nc.scalar.dma_start(
                out=h2T_fin[:].rearrange("p (r b) -> p r b", r=8),
                in_=agout1_prev[:].rearrange("(r p) b -> p r b", p=128))
            h2T_prev = h2T_fin
            pd = ps.tile([16, DC], fp32, tag="dec")
            for k in range(KH):
                nc.tensor.matmul(pd[:], h2T_prev[:, k * 16:(k + 1) * 16],
                                 decw[:, k * DC:(k + 1) * DC],
                                 start=(k == 0), stop=False)
            nc.tensor.matmul(pd[:], ones[0:1, 0:16], decb[:], start=False, stop=True)
            od = gp.tile([16, DC], fp32, tag="od")
            nc.vector.tensor_copy(od[:], pd[:])
            nc.sync.dma_start(out=out_d, in_=od[:])


    nc.compile()
    return nc


def _gate_rows(c):
    """Row indices (into 3H) owned by core c: r, z, n sections of its slice."""
    sl = np.arange(c * HC, (c + 1) * HC)
    return np.concatenate([sl, H + sl, 2 * H + sl])


def kernel(*a, **kw):
    out, _ = _run(False, *a, **kw)
    return out


def kernel_dbg(*a, **kw):
    return _run(True, *a, **kw)


def _run(debug, x, w_ih_l0, w_hh_l0, b_ih_l0, b_hh_l0,
         w_ih_l1, w_hh_l1, b_ih_l1, b_hh_l1, dec_w, dec_b):
    key = ("dbg" if debug else "nc")
    if key not in _CACHE:
        _CACHE[key] = _build(debug)
    nc = _CACHE[key]

    x = np.asarray(x, np.float32)
    # xT tiled: [MT, KX, 128, 128]; token index = t*16+b
    xT = np.ascontiguousarray(x.transpose(2, 1, 0).reshape(NN, T * B))
    xT_t = np.ascontiguousarray(
        xT.reshape(KX, 128, MT, 128).transpose(2, 1, 0, 3))

    def pack_kT(w_rows, kchunks, ncols):
        """w_rows [ncols_rows, K]: -> packed [128, kchunks*ncols] where
        packed[p, k*ncols+g] = w_rows[g, k*128+p] (i.e. w_rows.T chunks)."""
        wT = np.ascontiguousarray(np.asarray(w_rows, np.float32).T)  # [K, ncols]
        return np.ascontiguousarray(
            wT.reshape(kchunks, 128, ncols).transpose(1, 0, 2).reshape(128, kchunks * ncols))

    import ml_dtypes
    def to_bf16(a):
        return np.ascontiguousarray(a.astype(ml_dtypes.bfloat16))
    ones = np.ones((1, 128), np.float32)
    eye16 = np.eye(16, dtype=np.float32)

    in_maps = []
    for c in range(NC):
        rows = _gate_rows(c)
        drows = slice(c * DC, (c + 1) * DC)
        def pack_T(w_rows, kchunks):
            # [384, K] -> [128, kchunks*384], block (k,g) at col k*384+g*128:
            # [p, k*384+g*128+j] = w_rows[g*128+j, k*128+p]
            Wt = np.ascontiguousarray(np.asarray(w_rows, np.float32).T)
            return to_bf16(Wt.reshape(kchunks, 128, 3, 128)
                           .transpose(1, 0, 2, 3).reshape(128, kchunks * GC))
        m = {
            "xT": to_bf16(xT_t),
            "eye128": np.eye(128, dtype=np.float32),
            "wih0": pack_T(np.asarray(w_ih_l0)[rows], KX),
            "whh0": pack_T(np.asarray(w_hh_l0)[rows], KH),
            "wih1": to_bf16(pack_kT(np.asarray(w_ih_l1)[rows], KH, GC)),
            "whh1": to_bf16(pack_kT(np.asarray(w_hh_l1)[rows], KH, GC)),
            "decw": to_bf16(pack_kT(np.asarray(dec_w)[drows], KH, DC)),
            "bih0": np.asarray(b_ih_l0, np.float32)[rows][None, :],
            "bhh0": np.asarray(b_hh_l0, np.float32)[rows][None, :],
            "bih1": np.asarray(b_ih_l1, np.float32)[rows][None, :],
            "bhh1": np.asarray(b_hh_l1, np.float32)[rows][None, :],
            "decb": np.asarray(dec_b, np.float32)[drows][None, :],
            "ones": ones, "eye16": eye16,
        }
        in_maps.append(m)

    _CACHE["last_in_maps"] = in_maps
    res = bass_utils.run_bass_kernel_spmd(
        nc, in_maps, core_ids=list(range(NC)), trace=False)
    out = np.concatenate([res.results[c]["out"] for c in range(NC)], axis=1)
    return out, res



# revision 7
# speedup vs baseline: 2.1624x; 2.1624x over previous
"""Trainium2 Bass kernel for nn_BaselineGRU: 2-layer GRU (B=16,T=64,NN=4096,H=1024)
+ decoder, on 8 NeuronCores.

Strategy: gate-dimension sharding (each core owns a 128-wide hidden slice =
384 gate rows per layer), software-pipelined layers, one fused AllGather of
both layers' hidden-state chunks per pipeline slot.  All GEMMs keep the
stationary operand tiny (h.T chunks [128,16]) and stream weights as the
moving operand (fp32, N=384).  The layer-0 input projection is interleaved
into the PE stream so it hides under the early slots' AllGather windows.
"""
import numpy as np

import concourse.bacc as bacc
import concourse.tile as tile
import concourse.mybir as mybir
from concourse import bass_utils

B, T, NN, H = 16, 64, 4096, 1024
G3 = 3 * H                      # 3072 gate rows per layer
NC = 8                          # cores
HC = H // NC                    # 128 hidden per core
GC = 3 * HC                     # 384 gate rows per core
DC = NN // NC                   # 512 decoder rows per core
KH = H // 128                   # 8 K-chunks over hidden
KX = NN // 128                  # 32 K-chunks over input features
MT = (B * T) // 128             # 8 token blocks
fp32 = mybir.dt.float32
fp32r = mybir.dt.float32r
bf16 = mybir.dt.bfloat16       # PE: 1 cycle/moving-row for N>=256 (fp32: 4)

_CACHE = {}


def _build(debug=False, no_collective=False):
    nc = bacc.Bacc("TRN2", target_bir_lowering=False, debug=False,
                   enable_asserts=False, num_devices=NC)
    d = {}
    # ---- DRAM I/O ----
    d["xT"] = nc.dram_tensor("xT", [MT, 128, KX, 128], bf16, kind="ExternalInput").ap()
    d["wih0"] = nc.dram_tensor("wih0", [128, KX * GC], bf16, kind="ExternalInput").ap()
    d["whh0"] = nc.dram_tensor("whh0", [128, KH * GC], bf16, kind="ExternalInput").ap()
    d["wih1"] = nc.dram_tensor("wih1", [128, KH * GC], bf16, kind="ExternalInput").ap()
    d["whh1"] = nc.dram_tensor("whh1", [128, KH * GC], bf16, kind="ExternalInput").ap()
    d["decw"] = nc.dram_tensor("decw", [128, KH * DC], bf16, kind="ExternalInput").ap()
    d["smalls"] = nc.dram_tensor("smalls", [1, 4 * GC + DC + 128], fp32r,
                                 kind="ExternalInput").ap()
    d["eye16"] = nc.dram_tensor("eye16", [16, 16], fp32, kind="ExternalInput").ap()
    out_d = nc.dram_tensor("out", [B, DC], fp32, kind="ExternalOutput").ap()
    if debug:
        dbg_gx0 = nc.dram_tensor("dbg_gx0", [T, B, GC], fp32, kind="ExternalOutput").ap()
        dbg_ag0 = nc.dram_tensor("dbg_ag0", [2 * 128 * NC, 16], fp32, kind="ExternalOutput").ap()
        dbg_ag1 = nc.dram_tensor("dbg_ag1", [2 * 128 * NC, 16], fp32, kind="ExternalOutput").ap()

    S = mybir.ActivationFunctionType.Sigmoid
    TA = mybir.ActivationFunctionType.Tanh

    with tile.TileContext(nc) as tc:
        with tc.tile_pool(name="wsb", bufs=1) as wsb, \
             tc.tile_pool(name="xp", bufs=2) as xp, \
             tc.tile_pool(name="gxp", bufs=3) as gxp, \
             tc.tile_pool(name="hp", bufs=2) as hp, \
             tc.tile_pool(name="gp", bufs=2) as gp, \
             tc.tile_pool(name="ps", bufs=1, space="PSUM") as ps, \
             tc.tile_pool(name="dram", bufs=1, space="DRAM") as drp, \
             tc.tile_pool(name="agd", bufs=16, space="DRAM") as agd:

            # ---- persistent SBUF loads ----
            wih0 = wsb.tile([128, KX * GC], bf16, tag="wih0")
            whh0 = wsb.tile([128, KH * GC], bf16, tag="whh0")
            wih1 = wsb.tile([128, KH * GC], bf16, tag="wih1")
            whh1 = wsb.tile([128, KH * GC], bf16, tag="whh1")
            decw = wsb.tile([128, KH * DC], bf16, tag="decw")
            smalls = wsb.tile([1, 4 * GC + DC + 128], fp32r, tag="smalls")
            nc.sync.dma_start(out=smalls[:], in_=d["smalls"])
            bias = {nm: smalls[0:1, i * GC:(i + 1) * GC]
                    for i, nm in enumerate(("bih0", "bhh0", "bih1", "bhh1"))}
            decb = smalls[0:1, 4 * GC:4 * GC + DC]
            ones = smalls[0:1, 4 * GC + DC:4 * GC + DC + 128]
            eye16 = wsb.tile([16, 16], fp32, tag="eye16")
            nc.sync.dma_start(out=eye16[:], in_=d["eye16"])
            WQ = KX * GC // 4
            for q in range(4):
                nc.sync.dma_start(out=wih0[:, q * WQ:(q + 1) * WQ],
                                  in_=d["wih0"][:, q * WQ:(q + 1) * WQ])
            nc.sync.dma_start(out=whh0[:], in_=d["whh0"])
            nc.sync.dma_start(out=wih1[:], in_=d["wih1"])
            nc.sync.dma_start(out=whh1[:], in_=d["whh1"])
            nc.sync.dma_start(out=decw[:], in_=d["decw"])

            zero16 = wsb.tile([16, HC], fp32, tag="zero16")
            zeroT = wsb.tile([128, 16], fp32, tag="zeroT")
            nc.vector.memset(zeroT[:], 0.0)
            nc.vector.memset(zero16[:], 0.0)

            # gx0.T staging in DRAM: [MT, 3, 128 gates, 128 tokens]
            gx0_dram = drp.tile([MT, 3, 128, 128], fp32, tag="gx0d")

            # ---- layer-0 projection (transposed): out[g,tok] per gate-chunk
            def emit_proj_chunk(m):
                xtm = xp.tile([128, KX * 128], bf16, tag="xtm")
                nc.sync.dma_start(out=xtm[:].rearrange("p (k j) -> p k j", k=KX),
                                  in_=d["xT"][m])
                for g in range(3):
                    pp = ps.tile([128, 128], fp32, tag="proj")
                    for k in range(KX):
                        nc.tensor.matmul(pp[:],
                                         wih0[:, k * GC + g * 128:k * GC + (g + 1) * 128],
                                         xtm[:, k * 128:(k + 1) * 128],
                                         start=(k == 0), stop=False)
                    nc.tensor.matmul(pp[:], bias["bih0"][0:1, g * 128:(g + 1) * 128],
                                     ones[0:1, 0:128], start=False, stop=True)
                    psb = gxp.tile([128, 128], fp32, tag=f"projsb{g}")
                    psb_refs[(m, g)] = psb
                    nc.vector.tensor_copy(psb[:], pp[:])
                    nc.sync.dma_start(out=gx0_dram[m, g], in_=psb[:])

            # per-slot state
            h1T_prev = None      # [128, 128] SBUF: h1(s-1).T as 8 chunks of 16
            h2T_prev = None
            h1_old = zeroT       # [128,16] my slice of h1(s-1), transposed
            h2_old = zero16

            def gates(gh_ps, gx_ap, h_old, tagpfx):
                """gh_ps: PSUM [16,GC] (includes b_hh); gx_ap: [16,GC] (includes
                b_ih). Returns h_new [16,HC] SBUF tile."""
                pre = gp.tile([16, 2 * HC], fp32, tag=f"{tagpfx}pre")
                rz = gp.tile([16, 2 * HC], fp32, tag=f"{tagpfx}rz")
                c0 = gp.tile([16, HC], fp32, tag=f"{tagpfx}c0")
                d0 = gp.tile([16, HC], fp32, tag=f"{tagpfx}d0")
                n0 = gp.tile([16, HC], fp32, tag=f"{tagpfx}n0")
                e0 = gp.tile([16, HC], fp32, tag=f"{tagpfx}e0")
                f0 = gp.tile([16, HC], fp32, tag=f"{tagpfx}f0")
                hn = gp.tile([16, HC], fp32, tag=f"{tagpfx}hn")
                # pre_r,pre_z = gx[0:256] + gh[0:256]
                nc.vector.tensor_add(pre[:], gx_ap[:, 0:2 * HC], gh_ps[:, 0:2 * HC])
                nc.scalar.activation(rz[:], pre[:], S)
                # c = r * gh_n ; d = gx_n + c ; n = tanh(d)
                nc.vector.tensor_mul(c0[:], rz[:, 0:HC], gh_ps[:, 2 * HC:3 * HC])
                nc.vector.tensor_add(d0[:], gx_ap[:, 2 * HC:3 * HC], c0[:])
                nc.scalar.activation(n0[:], d0[:], TA)
                # h_new = n + z*(h_old - n)
                nc.vector.tensor_sub(e0[:], h_old[:], n0[:])
                nc.vector.tensor_mul(f0[:], rz[:, HC:2 * HC], e0[:])
                hnew = gp.tile([16, HC], fp32, tag=f"{tagpfx}hnew")
                nc.vector.tensor_add(hnew[:], n0[:], f0[:])
                return hnew

            proj_sched = {0: [0], 1: [1], 2: [2]}
            for j in range(5):
                proj_sched.setdefault(5 + j * 4, []).append(3 + j)
            psb_refs = {}

            for s in range(T + 1):
                for m in proj_sched.get(s, []):
                    emit_proj_chunk(m)

                # ---------- layer 0, step s (transposed gate layout) ----------
                if s < T:
                    # gx0.T for this step: [128, (g b)] = 3 gate-chunks x 16
                    if s == 0:
                        gxT = None
                        gx_sl = [psb_refs[(0, g)][:, 0:16] for g in range(3)]
                    else:
                        gxT = gxp.tile([128, 3 * 16], fp32, tag="gxT")
                        nc.sync.dma_start(
                            out=gxT[:].rearrange("p (g b) -> p g b", g=3),
                            in_=gx0_dram[s // 8, :, :,
                                         (s % 8) * 16:(s % 8) * 16 + 16]
                            .rearrange("g p b -> p g b"))
                        gx_sl = [gxT[:, g * 16:(g + 1) * 16] for g in range(3)]
                    # pre-activations assembled in PSUM: gh (+bhh) per chunk,
                    # plus gx injected for r,z (n keeps gx separate)
                    pch = []
                    for g in range(3):
                        pg = ps.tile([128, 16], fp32, tag=f"gh0T{g}")
                        first = True
                        if s > 0:
                            for k in range(KH):
                                nc.tensor.matmul(
                                    pg[:],
                                    whh0[:, k * GC + g * 128:k * GC + (g + 1) * 128],
                                    h1T_prev[:, k * 16:(k + 1) * 16],
                                    start=first, stop=False)
                                first = False
                        nc.tensor.matmul(pg[:],
                                         bias["bhh0"][0:1, g * 128:(g + 1) * 128],
                                         ones[0:1, 0:16],
                                         start=first, stop=True)
                        pch.append(pg)
                    prer = gp.tile([128, 16], fp32, tag="l0pr")
                    nc.vector.tensor_add(prer[:], gx_sl[0], pch[0][:])
                    prez = gp.tile([128, 16], fp32, tag="l0pz")
                    nc.vector.tensor_add(prez[:], gx_sl[1], pch[1][:])
                    r0 = gp.tile([128, 16], fp32, tag="l0r")
                    z0 = gp.tile([128, 16], fp32, tag="l0z")
                    nc.scalar.activation(r0[:], prer[:], S)
                    nc.scalar.activation(z0[:], prez[:], S)
                    c0 = gp.tile([128, 16], fp32, tag="l0c")
                    nc.vector.tensor_mul(c0[:], r0[:], pch[2][:])
                    d0 = gp.tile([128, 16], fp32, tag="l0d")
                    nc.vector.tensor_add(d0[:], gx_sl[2], c0[:])
                    n0 = gp.tile([128, 16], fp32, tag="l0n")
                    nc.scalar.activation(n0[:], d0[:], TA)
                    e0 = gp.tile([128, 16], fp32, tag="l0e")
                    nc.vector.tensor_sub(e0[:], h1_old[:], n0[:])
                    f0 = gp.tile([128, 16], fp32, tag="l0f")
                    nc.vector.tensor_mul(f0[:], z0[:], e0[:])
                    h1_new = gp.tile([128, 16], fp32, tag="l0h")
                    nc.vector.tensor_add(h1_new[:], n0[:], f0[:])
                    hbf = gp.tile([128, 16], bf16, tag="l0hb")
                    nc.vector.tensor_copy(hbf[:], h1_new[:])
                    agin0 = agd.tile([128, 16], bf16, tag="agin0")
                    nc.sync.dma_start(out=agin0[:], in_=hbf[:])
                    agout0 = agd.tile([128 * NC, 16], bf16, tag="agout0")
                    nc.gpsimd.collective_compute(
                        "AllGather", mybir.AluOpType.bypass,
                        replica_groups=[list(range(NC))],
                        ins=[agin0.opt()], outs=[agout0.opt()])
                    h1T = hp.tile([128, 128], bf16, tag="h1T")
                    nc.sync.dma_start(
                        out=h1T[:].rearrange("p (r b) -> p r b", r=8),
                        in_=agout0[:].rearrange("(r p) b -> p r b", p=128))
                    h1_old = h1_new

                # ---------- layer 1, step s-1 ----------
                if s >= 2:
                    # fetch h2(s-2).T gathered by last slot's AG1; its wait is
                    # already satisfied here, so no sequencer head-of-line block
                    h2T = hp.tile([128, 128], bf16, tag="h2T")
                    nc.scalar.dma_start(
                        out=h2T[:].rearrange("p (r b) -> p r b", r=8),
                        in_=agout1_prev[:].rearrange("(r p) b -> p r b", p=128))
                    h2T_prev = h2T
                if s >= 1:
                    gx1 = ps.tile([16, GC], fp32, tag="gx1")
                    for k in range(KH):
                        nc.tensor.matmul(gx1[:], h1T_prev[:, k * 16:(k + 1) * 16],
                                         wih1[:, k * GC:(k + 1) * GC],
                                         start=(k == 0), stop=False)
                    nc.tensor.matmul(gx1[:], ones[0:1, 0:16], bias["bih1"][:],
                                     start=False, stop=True)
                    gh1 = ps.tile([16, GC], fp32, tag="gh1")
                    if s == 1:
                        nc.tensor.matmul(gh1[:], ones[0:1, 0:16], bias["bhh1"][:],
                                         start=True, stop=True)
                    else:
                        for k in range(KH):
                            nc.tensor.matmul(gh1[:], h2T_prev[:, k * 16:(k + 1) * 16],
                                             whh1[:, k * GC:(k + 1) * GC],
                                             start=(k == 0), stop=False)
                        nc.tensor.matmul(gh1[:], ones[0:1, 0:16], bias["bhh1"][:],
                                         start=False, stop=True)
                    gx1s = gxp.tile([16, GC], fp32, tag="gx1s")
                    nc.vector.tensor_copy(gx1s[:], gx1[:])
                    h2_new = gates(gh1, gx1s[:], h2_old, "l1")
                    t1 = ps.tile([128, 16], fp32, tag="t1")
                    nc.tensor.transpose(t1[:], h2_new[:], eye16[:])
                    t1s = gp.tile([128, 16], bf16, tag="t1s")
                    nc.vector.tensor_copy(t1s[:], t1[:])
                    agin1 = agd.tile([128, 16], bf16, tag="agin1")
                    nc.scalar.dma_start(out=agin1[:], in_=t1s[:])
                    agout1 = agd.tile([128 * NC, 16], bf16, tag="agout1")
                    nc.gpsimd.collective_compute(
                        "AllGather", mybir.AluOpType.bypass,
                        replica_groups=[list(range(NC))],
                        ins=[agin1.opt()], outs=[agout1.opt()])
                    agout1_prev = agout1
                    h2_old = h2_new

                if s < T:
                    h1T_prev = h1T

            # ---------- decoder: out = h2(T-1) @ dec_w_c.T + dec_b_c ----------
            h2T_fin = hp.tile([128, 128], bf16, tag="h2T")
            nc.scalar.dma_start(
                out=h2T_fin[:].rearrange("p (r b) -> p r b", r=8),
                in_=agout1_prev[:].rearrange("(r p) b -> p r b", p=128))
            h2T_prev = h2T_fin
            pd = ps.tile([16, DC], fp32, tag="dec")
            for k in range(KH):
                nc.tensor.matmul(pd[:], h2T_prev[:, k * 16:(k + 1) * 16],
                                 decw[:, k * DC:(k + 1) * DC],
                                 start=(k == 0), stop=False)
            nc.tensor.matmul(pd[:], ones[0:1, 0:16], decb[:], start=False, stop=True)
            od = gp.tile([16, DC], fp32, tag="od")
            nc.vector.tensor_copy(od[:], pd[:])
            nc.sync.dma_start(out=out_d, in_=od[:])


    nc.compile()
    return nc


def _gate_rows(c):
    """Row indices (into 3H) owned by core c: r, z, n sections of its slice."""
    sl = np.arange(c * HC, (c + 1) * HC)
    return np.concatenate([sl, H + sl, 2 * H + sl])


def kernel(*a, **kw):
    out, _ = _run(False, *a, **kw)
    return out


def kernel_dbg(*a, **kw):
    return _run(True, *a, **kw)


def _run(debug, x, w_ih_l0, w_hh_l0, b_ih_l0, b_hh_l0,
         w_ih_l1, w_hh_l1, b_ih_l1, b_hh_l1, dec_w, dec_b):
    key = ("dbg" if debug else "nc")
    if key not in _CACHE:
        _CACHE[key] = _build(debug)
    nc = _CACHE[key]

    x = np.asarray(x, np.float32)
    # xT tiled: [MT, KX, 128, 128]; token index = t*16+b
    xT = np.ascontiguousarray(x.transpose(2, 1, 0).reshape(NN, T * B))
    xT_t = np.ascontiguousarray(
        xT.reshape(KX, 128, MT, 128).transpose(2, 1, 0, 3))

    def pack_kT(w_rows, kchunks, ncols):
        """w_rows [ncols_rows, K]: -> packed [128, kchunks*ncols] where
        packed[p, k*ncols+g] = w_rows[g, k*128+p] (i.e. w_rows.T chunks)."""
        wT = np.ascontiguousarray(np.asarray(w_rows, np.float32).T)  # [K, ncols]
        return np.ascontiguousarray(
            wT.reshape(kchunks, 128, ncols).transpose(1, 0, 2).reshape(128, kchunks * ncols))

    import ml_dtypes
    def to_bf16(a):
        return np.ascontiguousarray(a.astype(ml_dtypes.bfloat16))
    ones = np.ones((1, 128), np.float32)
    eye16 = np.eye(16, dtype=np.float32)

    in_maps = []
    for c in range(NC):
        rows = _gate_rows(c)
        drows = slice(c * DC, (c + 1) * DC)
        def pack_T(w_rows, kchunks):
            # [384, K] -> [128, kchunks*384], block (k,g) at col k*384+g*128:
            # [p, k*384+g*128+j] = w_rows[g*128+j, k*128+p]
            Wt = np.ascontiguousarray(np.asarray(w_rows, np.float32).T)
            return to_bf16(Wt.reshape(kchunks, 128, 3, 128)
                           .transpose(1, 0, 2, 3).reshape(128, kchunks * GC))
        m = {
            "xT": to_bf16(xT_t),
            "wih0": pack_T(np.asarray(w_ih_l0)[rows], KX),
            "whh0": pack_T(np.asarray(w_hh_l0)[rows], KH),
            "wih1": to_bf16(pack_kT(np.asarray(w_ih_l1)[rows], KH, GC)),
            "whh1": to_bf16(pack_kT(np.asarray(w_hh_l1)[rows], KH, GC)),
            "decw": to_bf16(pack_kT(np.asarray(dec_w)[drows], KH, DC)),
            "smalls": np.concatenate(
                [np.asarray(b_ih_l0, np.float32)[rows][None, :],
                 np.asarray(b_hh_l0, np.float32)[rows][None, :],
                 np.asarray(b_ih_l1, np.float32)[rows][None, :],
                 np.asarray(b_hh_l1, np.float32)[rows][None, :],
                 np.asarray(dec_b, np.float32)[drows][None, :],
                 ones], axis=1),
            "eye16": eye16,
        }
        in_maps.append(m)

    _CACHE["last_in_maps"] = in_maps
    res = bass_utils.run_bass_kernel_spmd(
        nc, in_maps, core_ids=list(range(NC)), trace=False)
    out = np.concatenate([res.results[c]["out"] for c in range(NC)], axis=1)
    return out, res

